# revision 1
# baseline (speedup 1.0000x reference)
"""DUPLEX GAT on trn2 — v2: bf16 phase-B, interleaved g table, node-major scatter.

Design (per core, SPMD over 8 cores; nodes permuted into nw=392 windows of 128,
rotated per core so own dst windows are rows [0, wpc*128) of its table):
  - Phase A: per window w, per stack s: g_ps = xw_s.T @ Wg_s where
    Wg_s = [W@T_bd | W@R_bd] (132 cols: ft in el-carrying basis + er(4)).
    ft cols -> interleaved bf16 DRAM table row [am(128) | ph(128)] (512B rows);
    er cols -> SBUF erw tile (own windows only).
  - Phase B per dst window: one dma_gather per index group (A: rows<32768 of
    the rotated table, B: rest) pulls [128, tt, 256] bf16 (both stacks per row).
    S one-hot (edges x nodes) built bf16 on DVE via iota==dmb.
    ST = PE transpose of S tiles (bf16 PSUM) -> Act copy to SBUF.
    er per edge: er_ps[:,t,:] = ST_t.T @ erw_w  (PSUM f32).
    tb = el (strided Z cols) + er; lrelu (DVE); p = exp (Act, bf16).
    cb = p * w_edge; Z *= cb (per 32-block).
    num_ps [128 nodes, 256] += S_t.T @ Z_t ; den_ps [128, 8] += S_t.T @ p_t.
    Epilogue: rec = 1/max(den,eps); sca = num*rec (bf16); per stack PE
    transpose -> [feat, nodes]; h = Tinv.T @ sca; bias + relu/identity; out.
"""
import sys
sys.path.insert(0, '/opt/trn_rl_repo')
from dataclasses import dataclass

import numpy as np
import ml_dtypes

import concourse.bass as bass
import concourse.bacc as bacc
import concourse.tile as tile
from concourse import mybir

F32 = mybir.dt.float32
BF16 = mybir.dt.bfloat16
I16 = mybir.dt.int16
P = 128
NEG = 0.2
BF = ml_dtypes.bfloat16


def _patch_drain_split():
    import bass_rust
    from concourse.tile import ScopedClock
    if getattr(tile.TileContext, "_drain_patched_v2", False):
        return

    def patched(self, tick_clock, wait_clock):
        nc = self.nc
        drain_inst = nc.sync.drain()
        wait_clock.add_sem_waits(
            drain_inst.ins, ScopedClock({None: tick_clock.global_clock}))
        si = drain_inst.ins.sync_info
        waits = list(si.on_wait) if si is not None else []
        if len(waits) > 1:
            si.on_wait = waits[:1]
            for i in range(1, len(waits)):
                d2 = nc.sync.drain()
                d2.ins.sync_info = bass_rust.SyncInfo(
                    on_wait=waits[i : i + 1], on_update=[])
        nc.all_engine_barrier()
        popped = nc._tile_sem_poison_stack.pop()
        assert popped is self._sem_poison
        nc.clear_and_free_semaphores(list(self.sems.allocated().values()))
        nc.all_engine_barrier()

    tile.TileContext._drain_and_barrier = patched
    tile.TileContext._drain_patched_v2 = True


_patch_drain_split()


def _patch_loud_ncc():
    import traceback
    from concourse import bass2jax
    if getattr(bass2jax, "_loud_ncc", False):
        return
    bass2jax._loud_ncc = True
    orig = bass2jax.neuronx_cc_hook

    def logged(*a, **k):
        try:
            return orig(*a, **k)
        except BaseException:
            with open("/tmp/ncc_hook_err.log", "a") as f:
                f.write(traceback.format_exc() + "\n")
            raise

    bass2jax.neuronx_cc_hook = logged


_patch_loud_ncc()


@dataclass
class Cfg:
    n_nodes: int = 50000
    n_edges: int = 800000
    n_cores: int = 8
    wpc: int = 49
    split_w: int = 255    # windows in table A (A rows = 255*128 + 128 zero)
    heads: int = 4
    fdim: int = 32
    in_dim: int = 128
    gbatch: int = 8         # tiles per dma_gather call (HW desc-ring limit)

    @property
    def nw(self):
        return self.n_cores * self.wpc

    @property
    def n_pad(self):
        return self.nw * P


# ----------------------------------------------------------------- host prep

def balance_windows(deg, cfg):
    import heapq
    n_pad, nw = cfg.n_pad, cfg.nw
    degp = np.zeros(n_pad, np.int64)
    degp[: len(deg)] = deg
    order = np.argsort(-degp, kind="stable")
    heap = [(0, w, 0) for w in range(nw)]
    heapq.heapify(heap)
    win_of = np.empty(n_pad, np.int32)
    slot_of = np.empty(n_pad, np.int32)
    for node in order:
        load, w, cnt = heapq.heappop(heap)
        win_of[node] = w
        slot_of[node] = cnt
        cnt += 1
        if cnt < P:
            heapq.heappush(heap, (load + degp[node], w, cnt))
    return win_of.astype(np.int64) * P + slot_of


def wrap_idx(idx):
    n = len(idx)
    blk = np.asarray(idx, np.int16).reshape(n // 16, 16).T
    return np.tile(blk, (8, 1))


def prep_graph(src, dst, cfg):
    """Global (unrotated) node order. Gather tables split at SPLIT_W windows:
    gtabA = rows [0, 32640) + 128 zero rows; gtabB = rest + 128 zero rows.
    Each window gets two 128-idx "extra" tiles gathering its own dst rows
    (one real, the other pointing at the zero rows)."""
    c = cfg
    SPLIT_W = c.split_w                      # 255 windows -> 32640 rows
    NAR = SPLIT_W * P                        # real rows in A
    deg = np.bincount(dst, minlength=c.n_nodes)
    new_id = balance_windows(deg, c)
    node_at = np.full(c.n_pad, -1, np.int64)
    node_at[new_id] = np.arange(c.n_pad)

    src_n = new_id[src]
    dst_n = new_id[dst]
    order = np.argsort(dst_n, kind="stable")
    src_s, dst_s = src_n[order], dst_n[order]
    win_s = dst_s // P
    bounds = np.searchsorted(win_s, np.arange(c.nw + 1))

    TA = TB = 0
    for g in range(c.nw):
        lo, hi = bounds[g], bounds[g + 1]
        rr = src_s[lo:hi]
        nA = int((rr < NAR).sum())
        nB = int((hi - lo) - nA)
        TA = max(TA, -(-nA // P))
        TB = max(TB, -(-nB // P))
    TA = max(TA, 1)
    TB = max(TB, 1)
    T = TA + TB

    nco, wpc = c.n_cores, c.wpc
    # idx layout (T+2 tiles): [extraA | A reals (TA) | extraB | B reals (TB)]
    idx16 = np.zeros((nco, wpc, P, (T + 2) * 8), np.int16)
    dmb = np.full((nco, wpc, P, T), -1000.0, np.float32)
    epos = np.full((nco, wpc, T * P), -1, np.int64)

    zA = NAR + np.arange(P)                  # dedicated zero rows in A
    zB = (c.n_pad - NAR) + np.arange(P)      # dedicated zero rows in B
    for core in range(nco):
        for wl in range(wpc):
            g = core * wpc + wl
            lo, hi = bounds[g], bounds[g + 1]
            rr = src_s[lo:hi]
            mA = rr < NAR
            iA = np.where(mA)[0]
            iB = np.where(~mA)[0]
            la = np.zeros(TA * P, np.int64)
            lb = np.zeros(TB * P, np.int64)
            la[: len(iA)] = rr[iA]
            lb[: len(iB)] = rr[iB] - NAR
            # own dst rows (128), real in the region of this window
            own = g * P + np.arange(P)
            if g < SPLIT_W:
                exA, exB = own, zB
            else:
                exA, exB = zA, own - NAR
            idx16[core, wl, :, 0:8] = wrap_idx(exA)
            idx16[core, wl, :, 8 : (TA + 1) * 8] = wrap_idx(la)
            idx16[core, wl, :, (TA + 1) * 8 : (TA + 2) * 8] = wrap_idx(exB)
            idx16[core, wl, :, (TA + 2) * 8 :] = wrap_idx(lb)
            dv = np.full(T * P, -1000.0, np.float32)
            dv[: len(iA)] = (dst_s[lo:hi][iA] % P).astype(np.float32)
            dv[TA * P : TA * P + len(iB)] = (dst_s[lo:hi][iB] % P).astype(np.float32)
            dmb[core, wl] = dv.reshape(T, P).T
            ep = np.full(T * P, -1, np.int64)
            ep[: len(iA)] = lo + iA
            ep[TA * P : TA * P + len(iB)] = lo + iB
            epos[core, wl] = ep

    return dict(new_id=new_id, node_at=node_at, order=order,
                TA=TA, TB=TB, idx16=idx16, dmb=dmb, epos=epos)


def edge_w_tables(g, w_edge, cfg):
    """(nc, wpc, P, T) f32 edge weights in tile layout."""
    ws = np.asarray(w_edge)[g["order"]]
    ep = g["epos"]
    out = np.where(ep >= 0, ws[np.clip(ep, 0, None)], 0.0).astype(np.float32)
    n, w, TP = out.shape
    T = TP // P
    return out.reshape(n, w, T, P).transpose(0, 1, 3, 2).copy()


def pack_dmf(g, w_am, w_ph, cfg):
    """[nc, wpc, 128, 3T] bf16: [0:T) dmb, then per t: (w_am, w_ph)."""
    nco, wpc, _, T = g["dmb"].shape
    dmf = np.zeros((nco, wpc, P, 3 * T), np.float32)
    dmf[..., 0:T] = g["dmb"]
    dmf[..., T::2] = w_am
    dmf[..., T + 1 :: 2] = w_ph
    return dmf.astype(BF)


def prep_conv(W, al, ar, b, mean_heads, cfg):
    H, F = cfg.heads, cfg.fdim
    W = np.asarray(W, np.float64)
    al = np.asarray(al, np.float64)
    ar = np.asarray(ar, np.float64)
    b = np.asarray(b, np.float64)
    T_bd = np.zeros((H * F, H * F))
    for h in range(H):
        a = al[h]
        M = np.concatenate([a[:, None], np.eye(F)[:, : F - 1]], 1)
        Q, _ = np.linalg.qr(M)
        blk = np.concatenate([a[:, None], Q[:, 1:]], 1)
        T_bd[h * F : (h + 1) * F, h * F : (h + 1) * F] = blk
    Tinv = np.linalg.inv(T_bd)
    R_bd = np.zeros((H * F, H))
    for h in range(H):
        R_bd[h * F : (h + 1) * F, h] = ar[h]
    # f-major column order for the ft block: new col f*H + h <- old h*F + f.
    Wft = (W @ T_bd).reshape(-1, H, F).transpose(0, 2, 1).reshape(-1, H * F)
    out = dict(Wg=np.ascontiguousarray(Wft).astype(BF))
    # er from stored (bd, f-major) g rows: er = g_fm @ AR_fm, AR_fm rows f-major
    AR = Tinv @ R_bd                                   # (HF, H), rows bd order
    AR_fm = AR.reshape(H, F, H).transpose(1, 0, 2).reshape(H * F, H)
    out["AR"] = np.ascontiguousarray(AR_fm).astype(BF)
    Tinv_fm = Tinv.reshape(H, F, H * F).transpose(1, 0, 2).reshape(H * F, H * F)
    if mean_heads:
        Mm = np.zeros((H * F, F))
        for h in range(H):
            Mm[h * F : (h + 1) * F] = np.eye(F) / H
        out["Tinv"] = (Tinv_fm @ Mm).astype(BF)
        out["bcol"] = (b.reshape(H, F).mean(0))[:, None].astype(np.float32)
    else:
        out["Tinv"] = Tinv_fm.astype(BF)
        out["bcol"] = b[:, None].astype(np.float32)
    return out


def consts_np(cfg, T):
    iota3 = np.tile(np.arange(P, dtype=np.float32)[None, :, None],
                    (P, 1, T)).astype(BF)
    ident = np.eye(P, dtype=np.float32).astype(BF)
    return dict(iota3=iota3, ident=ident)


def pack_meta(g, dmf):
    """[nc, wpc, 128, T*8 + 3T] int16: gather idxs then dmf bits."""
    return np.concatenate([g["idx16"], dmf.view(np.int16)], axis=-1)


def to_xT_tiled(x, g, cfg):
    """x (n_nodes, D) -> permuted transposed tiles (nw, D, 128) f32."""
    n_pad = cfg.n_pad
    D = x.shape[1]
    xp = np.zeros((n_pad, D), np.float32)
    real = g["node_at"] >= 0
    idx = g["node_at"][real]
    keep = idx < cfg.n_nodes
    xp[np.where(real)[0][keep]] = np.asarray(x, np.float32)[idx[keep]]
    return np.ascontiguousarray(xp.reshape(cfg.nw, P, D).transpose(0, 2, 1))


def rotate_flat_bf16(x_tiled, core, cfg):
    """(nw, D, 128) -> rotated, [D, nw*128] bf16 for `core`."""
    rot = np.roll(np.arange(cfg.nw), -core * cfg.wpc)
    r = x_tiled[rot]                       # (nw, D, 128)
    D = r.shape[1]
    return np.ascontiguousarray(
        r.transpose(1, 0, 2).reshape(D, cfg.nw * P)).astype(BF)


# ------------------------------------------------------------ fused program

def build_fused(cfg, TA, TB, world1=False):
    """Both GAT layers in one program; h exchanged via AllGather."""
    c = cfg
    T = TA + TB
    HF = c.heads * c.fdim            # 128
    NAR = c.split_w * P              # real A rows
    NA = NAR + P                     # incl zero rows
    NB = (c.n_pad - NAR) + P
    nc = bacc.Bacc("TRN2", target_bir_lowering=False, debug=False)

    def dram_in(name, shape, dt=BF16):
        return nc.dram_tensor(name, list(shape), dt, kind="ExternalInput")

    if world1:
        xT2 = dram_in("xT2", (2, c.in_dim, c.nw * P))
    else:
        xin = dram_in("xin", (c.wpc, c.in_dim, 2, P))
    Wg = {}
    Tinv = {}
    bcol = {}
    AR = {}
    for li in (0, 1):
        for s in ("am", "ph"):
            Wg[(li, s)] = dram_in(f"Wg{li}_{s}", (c.in_dim, HF))
            oc = c.fdim if li else HF
            Tinv[(li, s)] = dram_in(f"Tinv{li}_{s}", (HF, oc))
            bcol[(li, s)] = dram_in(f"bcol{li}_{s}", (oc, 1), F32)
            AR[(li, s)] = dram_in(f"AR{li}_{s}", (HF, c.heads))
    iota3_d = dram_in("iota3", (P, P, T))
    ident_d = dram_in("ident", (P, P))
    meta_d = dram_in("meta", (c.wpc, P, (T + 2) * 8 + 3 * T), I16)
    hext_d = dram_in("hext", (c.nw, HF, 2, P)) if world1 else None

    out_d = nc.dram_tensor("out2", [c.wpc, c.fdim, 2, P], F32,
                           kind="ExternalOutput")
    stacks = ("am", "ph")
    CH = 8
    SC = 6
    n_chunk = -(-T // SC)

    with tile.TileContext(nc) as tc:
        with (
            tc.tile_pool(name="dram", bufs=1, space="DRAM") as dpool,
            tc.tile_pool(name="const", bufs=1) as cpool,
        ):
            gt = {}
            for li in (0, 1):
                gt[(li, "A")] = dpool.tile([NA, 2 * HF], BF16,
                                           name=f"gtab{li}A", tag=f"g{li}A")
                gt[(li, "B")] = dpool.tile([NB, 2 * HF], BF16,
                                           name=f"gtab{li}B", tag=f"g{li}B")
            hloc = dpool.tile([c.wpc, HF, 2, P], BF16, name="hloc", tag="hloc")
            if not world1:
                xloc = dpool.tile([c.wpc, c.in_dim, 2, P], BF16, name="xloc",
                                  tag="xloc")
                xfull = dpool.tile([c.nw, c.in_dim, 2, P], BF16, name="xfull",
                                   tag="xfull", addr_space="Shared")
            if world1:
                hfull = None
            else:
                hfull = dpool.tile([c.nw, HF, 2, P], BF16, name="hfull",
                                   tag="hfull", addr_space="Shared")

            ct = {}
            t = cpool.tile([P, P, T], BF16, name="ct_iota3")
            nc.sync.dma_start(t[:], iota3_d[:])
            ct["iota3"] = t
            t = cpool.tile([P, P], BF16, name="ct_ident")
            nc.sync.dma_start(t[:], ident_d[:])
            ct["ident"] = t
            for li in (0, 1):
                for s in stacks:
                    for nm, hd, dt in [("Wg", Wg[(li, s)], BF16),
                                       ("Tinv", Tinv[(li, s)], BF16),
                                       ("bcol", bcol[(li, s)], F32),
                                       ("AR", AR[(li, s)], BF16)]:
                        tt_ = cpool.tile(list(hd.shape), dt,
                                         name=f"ct_{nm}{li}_{s}")
                        nc.sync.dma_start(tt_[:], hd[:])
                        ct[(nm, li, s)] = tt_
            zrow = cpool.tile([P, 2 * HF], BF16, name="zrow")
            nc.vector.memset(zrow[:], 0.0)

            regs = {}

            def _reg(n):
                if n not in regs:
                    regs[n] = nc.gpsimd.to_reg(n)
                return regs[n]

            def phase_a(li, src_getter):
                """src_getter(w0) -> AP for [128, 2, CH, 128] bf16 load."""
                gtabA, gtabB = gt[(li, "A")], gt[(li, "B")]
                # zero rows
                nc.sync.dma_start(
                    gtabA[NAR : NAR + P, :].rearrange("(w p) d -> p w d", p=P),
                    zrow[:].unsqueeze(1))
                nc.sync.dma_start(
                    gtabB[NB - P : NB, :].rearrange("(w p) d -> p w d", p=P),
                    zrow[:].unsqueeze(1))
                with (
                    tc.tile_pool(name=f"pa_x{li}", bufs=3) as pax,
                    tc.tile_pool(name=f"pa_g{li}", bufs=3) as pag,
                    tc.tile_pool(name=f"pa_ps{li}", bufs=4, space="PSUM") as paps,
                ):
                    for w0 in range(0, c.nw, CH):
                        xw = pax.tile([c.in_dim, 2, CH, P], BF16,
                                      name="xw", tag="x")
                        nc.sync.dma_start(xw[:], src_getter(w0))
                        gsb = pag.tile([P, CH, 2 * HF], BF16,
                                       name="gsb", tag="gsb")
                        for k in range(CH):
                            g_ps = paps.tile([P, 2, HF], F32,
                                             name="g_ps", tag="g")
                            for si, s in enumerate(stacks):
                                nc.tensor.matmul(
                                    g_ps[:, si, :], xw[:, si, k, :],
                                    ct[("Wg", li, s)][:],
                                    start=True, stop=True)
                            dst = gsb[:, k, :].rearrange(
                                "p (f s h) -> p s f h", s=2, h=c.heads)
                            srcv = g_ps[:].rearrange(
                                "p s (f h) -> p s f h", h=c.heads)
                            if (w0 + k) % 2 == 0:
                                nc.scalar.copy(dst, srcv)
                            else:
                                nc.vector.tensor_copy(dst, srcv)
                        if (w0 + CH) * P <= NAR:
                            dst_ap = gtabA[w0 * P : (w0 + CH) * P, :]
                        elif w0 * P >= NAR:
                            dst_ap = gtabB[w0 * P - NAR : (w0 + CH) * P - NAR, :]
                        else:
                            # chunk straddles the split: two writes
                            kk = (NAR - w0 * P) // P
                            nc.sync.dma_start(
                                gtabA[w0 * P : NAR, :]
                                .rearrange("(w p) d -> p w d", p=P),
                                gsb[:, 0:kk, :])
                            nc.sync.dma_start(
                                gtabB[0 : (w0 + CH) * P - NAR, :]
                                .rearrange("(w p) d -> p w d", p=P),
                                gsb[:, kk:CH, :])
                            continue
                        nc.sync.dma_start(
                            dst_ap.rearrange("(w p) d -> p w d", p=P), gsb[:])

            def phase_b(li, pools):
                (pbm, pbz, pbs, pbst, pbsm, pbepi,
                 pst, pser, psacc, psepi, psden, psmm8) = pools
                gtabA, gtabB = gt[(li, "A")], gt[(li, "B")]
                last = li == 1
                OC = c.fdim if last else HF
                ODT = F32 if last else BF16
                # Z slots: [exA | A reals (TA) | exB | B reals (TB)]
                SA0, SB0 = 1, TA + 2           # first real slot per group
                st_meta = {}
                st_z = {}
                st_s = {}
                st_er = {}
                st_pz = {}
                st_acc = {}

                def slot(t):
                    return SA0 + t if t < TA else SB0 + (t - TA)

                def s0(wl):
                    meta_t = pbm.tile([P, (T + 2) * 8 + 3 * T], I16,
                                      name="meta", tag="meta")
                    nc.sync.dma_start(meta_t[:], meta_d[wl])
                    Z = pbz.tile([P, T + 2, 2 * HF], BF16, name="Z", tag="Z")
                    gb = c.gbatch
                    for base, ntile, tab in ((0, TA + 1, gtabA),
                                             (TA + 1, TB + 1, gtabB)):
                        for off in range(0, ntile, gb):
                            nb = min(gb, ntile - off)
                            o = base + off
                            nc.gpsimd.dma_gather(
                                out_ap=Z[:, o : o + nb, :], in_ap=tab[:],
                                idxs_ap=meta_t[:, o * 8 : (o + nb) * 8],
                                num_idxs=nb * P, num_idxs_reg=_reg(nb * P),
                                elem_size=2 * HF)
                    st_meta[wl] = meta_t
                    st_z[wl] = Z

                def s1(wl):
                    meta_t = st_meta[wl]
                    Z = st_z[wl]
                    dmf_t = meta_t[:, (T + 2) * 8 :].bitcast(BF16)
                    S = pbs.tile([P, P, T], BF16, name="S", tag="S")
                    nc.vector.tensor_tensor(
                        out=S[:],
                        in0=ct["iota3"][:],
                        in1=dmf_t[:, 0:T].unsqueeze(1).broadcast_to([P, P, T]),
                        op=mybir.AluOpType.is_equal)
                    # own-window dst rows -> erw (node-partitioned er)
                    ftw = pbsm.tile([P, 2, HF], BF16, name="ftw", tag="ftw")
                    nc.vector.tensor_tensor(
                        out=ftw[:].rearrange("p s (f h) -> p s f h",
                                             h=c.heads),
                        in0=Z[:, 0, :].rearrange("p (f s h) -> p s f h",
                                                 s=2, h=c.heads),
                        in1=Z[:, TA + 1, :].rearrange("p (f s h) -> p s f h",
                                                      s=2, h=c.heads),
                        op=mybir.AluOpType.add)
                    # ftwT[:, si, :] = window dst-node fts, feat-major; then
                    # erw[n, s*4+h] = ftwT_s.T @ AR_s  (node-partitioned er)
                    ftwT = pbsm.tile([P, 2, P], BF16, name="ftwT", tag="ftwT")
                    for si in range(2):
                        tr = pst.tile([P, SC, P], BF16, name="st_ps", tag="st")
                        nc.tensor.transpose(tr[:, 0, :], ftw[:, si, :],
                                            ct["ident"][:])
                        nc.scalar.copy(ftwT[:, si, :], tr[:, 0, :])
                    erw_ps = psmm8.tile([P, P], F32, name="mm8", tag="mm8")
                    for si, s in enumerate(stacks):
                        nc.tensor.matmul(
                            erw_ps[:, si * c.heads : (si + 1) * c.heads],
                            ftwT[:, si, :], ct[("AR", li, s)][:],
                            start=True, stop=True)
                    erw = pbsm.tile([P, 2 * c.heads], BF16, name="erw",
                                    tag="erw")
                    nc.scalar.copy(erw[:], erw_ps[:, 0 : 2 * c.heads])
                    # ST via PE transpose, chunked
                    STb = pbst.tile([P, T, P], BF16, name="ST", tag="ST")
                    for ch in range(n_chunk):
                        t0 = ch * SC
                        t1 = min(T, t0 + SC)
                        st_ps = pst.tile([P, SC, P], BF16, name="st_ps",
                                         tag="st")
                        for t in range(t0, t1):
                            nc.tensor.transpose(
                                st_ps[:, t - t0, :], S[:, :, t],
                                ct["ident"][:])
                        nc.scalar.copy(STb[:, t0:t1, :],
                                       st_ps[:, 0 : t1 - t0, :])
                    er_ps = pser.tile([P, T, 2 * c.heads], F32,
                                      name="er_ps", tag="erps")
                    for t in range(T):
                        nc.tensor.matmul(er_ps[:, t, :], STb[:, t, :],
                                         erw[:], start=True, stop=True)
                    st_s[wl] = S
                    st_er[wl] = er_ps

                def s2(wl):
                    Z = st_z[wl]
                    er_ps = st_er[wl]
                    meta_t = st_meta[wl]
                    dmf_t = meta_t[:, (T + 2) * 8 :].bitcast(BF16)
                    tb = pbsm.tile([P, T, 2 * c.heads], F32, name="tb",
                                   tag="tb")
                    Zc = Z[:].rearrange("p t (f c) -> p t f c",
                                        c=2 * c.heads)
                    nc.vector.tensor_tensor(
                        out=tb[:, 0:TA, :],
                        in0=Zc[:, SA0 : SA0 + TA, 0:1, :].squeeze(2),
                        in1=er_ps[:, 0:TA, :], op=mybir.AluOpType.add)
                    nc.vector.tensor_tensor(
                        out=tb[:, TA:T, :],
                        in0=Zc[:, SB0 : SB0 + TB, 0:1, :].squeeze(2),
                        in1=er_ps[:, TA:T, :], op=mybir.AluOpType.add)
                    nc.vector.scalar_tensor_tensor(
                        out=tb[:], in0=tb[:], scalar=NEG, in1=tb[:],
                        op0=mybir.AluOpType.mult, op1=mybir.AluOpType.max)
                    pz = pbsm.tile([P, T, 2 * c.heads], BF16, name="pz",
                                   tag="pz")
                    nc.scalar.activation(pz[:], tb[:],
                                         mybir.ActivationFunctionType.Exp)
                    cb = pbsm.tile([P, T, 2 * c.heads], BF16, name="cb",
                                   tag="cb")
                    nc.vector.tensor_tensor(
                        out=cb[:].rearrange("p t (s h) -> p t s h", s=2),
                        in0=pz[:].rearrange("p t (s h) -> p t s h", s=2),
                        in1=dmf_t[:, T : 3 * T]
                            .rearrange("p (t s) -> p t s", s=2)
                            .unsqueeze(3).broadcast_to([P, T, 2, c.heads]),
                        op=mybir.AluOpType.mult)
                    for lo, hi, s0_ in ((0, TA, SA0), (TA, T, SB0)):
                        nc.vector.tensor_tensor(
                            out=Zc[:, s0_ : s0_ + hi - lo, :, :],
                            in0=Zc[:, s0_ : s0_ + hi - lo, :, :],
                            in1=cb[:, lo:hi, :].unsqueeze(2)
                                .broadcast_to([P, hi - lo, c.fdim,
                                               2 * c.heads]),
                            op=mybir.AluOpType.mult)
                    st_pz[wl] = pz

                def s3(wl):
                    Z = st_z[wl]
                    S = st_s[wl]
                    pz = st_pz[wl]
                    num_t = psacc.tile([P, 2 * HF], F32, name="num", tag="num")
                    den_t = psden.tile([P, 2 * c.heads], F32, name="den",
                                       tag="den")
                    for t in range(T):
                        nc.tensor.matmul(num_t[:], S[:, :, t],
                                         Z[:, slot(t), :],
                                         start=(t == 0), stop=(t == T - 1))
                        nc.tensor.matmul(den_t[:], S[:, :, t], pz[:, t, :],
                                         start=(t == 0), stop=(t == T - 1))
                    st_acc[wl] = (num_t, den_t)

                def s4(wl):
                    num_t, den_t = st_acc.pop(wl)
                    num_ps = num_t[:]
                    den_ps = den_t[:]
                    denm = pbepi.tile([P, 2 * c.heads], F32, name="denm",
                                      tag="denm")
                    nc.vector.tensor_scalar(
                        out=denm[:], in0=den_ps, scalar1=1e-9, scalar2=None,
                        op0=mybir.AluOpType.max)
                    rec = pbepi.tile([P, 2 * c.heads], F32, name="rec",
                                     tag="rec")
                    nc.vector.reciprocal(rec[:], denm[:])
                    sca = pbepi.tile([P, 2, HF], BF16, name="sca", tag="sca")
                    numv = num_ps.rearrange("p (f s h) -> p f s h",
                                            s=2, h=c.heads)
                    recv = rec[:].rearrange("p (s h) -> p s h", s=2)
                    h2 = pbepi.tile([OC, 2, P], ODT, name="h2", tag="h2")
                    for si, s in enumerate(stacks):
                        nc.vector.tensor_tensor(
                            out=sca[:, si, :].rearrange("p (f h) -> p f h",
                                                        h=c.heads),
                            in0=numv[:, :, si, :],
                            in1=recv[:, si : si + 1, :]
                                .broadcast_to([P, c.fdim, c.heads]),
                            op=mybir.AluOpType.mult)
                        tr_ps = pst.tile([P, SC, P], BF16, name="st_ps",
                                         tag="st")
                        nc.tensor.transpose(tr_ps[:, 0, :], sca[:, si, :],
                                            ct["ident"][:])
                        scT = pbepi.tile([P, P], BF16, name="scT", tag="scT")
                        nc.scalar.copy(scT[:], tr_ps[:, 0, :])
                        h_ps = psmm8.tile([P, P], F32, name="mm8", tag="mm8")
                        nc.tensor.matmul(h_ps[0:OC, :],
                                         ct[("Tinv", li, s)][:], scT[:],
                                         start=True, stop=True)
                        nc.scalar.activation(
                            h2[:, si, :], h_ps[0:OC, :],
                            (mybir.ActivationFunctionType.Identity if last
                             else mybir.ActivationFunctionType.Relu),
                            bias=ct[("bcol", li, s)][:], scale=1.0)
                    if last:
                        nc.sync.dma_start(out_d[wl], h2[:])
                    else:
                        nc.sync.dma_start(hloc[wl], h2[:])

                s0(0)
                for i in range(c.wpc + 2):
                    if i + 1 < c.wpc:
                        s0(i + 1)
                    if i < c.wpc:
                        s1(i)
                    if 1 <= i <= c.wpc:
                        s2(i - 1)
                        s3(i - 1)
                    if i >= 2:
                        s4(i - 2)

            def mk_pools(li):
                return (
                    tc.tile_pool(name=f"pb{li}_meta", bufs=6),
                    tc.tile_pool(name=f"pb{li}_z", bufs=5),
                    tc.tile_pool(name=f"pb{li}_s", bufs=5),
                    tc.tile_pool(name=f"pb{li}_st", bufs=4),
                    tc.tile_pool(name=f"pb{li}_small", bufs=4),
                    tc.tile_pool(name=f"pb{li}_epi", bufs=3),
                    tc.tile_pool(name=f"ps{li}_t", bufs=2, space="PSUM"),
                    tc.tile_pool(name=f"ps{li}_er", bufs=2, space="PSUM"),
                    tc.tile_pool(name=f"ps{li}_acc", bufs=2, space="PSUM"),
                    tc.tile_pool(name=f"ps{li}_epi", bufs=1, space="PSUM"),
                    tc.tile_pool(name=f"ps{li}_den", bufs=1, space="PSUM"),
                    tc.tile_pool(name=f"ps{li}_mm8", bufs=1, space="PSUM"),
                )

            # ---- layer 0 ----
            if world1:
                phase_a(0, lambda w0: xT2[:, :, w0 * P : (w0 + CH) * P]
                        .rearrange("s d (w p) -> d s w p", p=P))
            else:
                nc.sync.dma_start(xloc[:], xin[:])
                nc.gpsimd.collective_compute(
                    "AllGather", mybir.AluOpType.bypass,
                    ins=[xloc[:]], outs=[xfull[:]],
                    replica_groups=[list(range(c.n_cores))])
                phase_a(0, lambda w0: xfull[w0 : w0 + CH]
                        .rearrange("w d s p -> d s w p"))
            import contextlib
            with contextlib.ExitStack() as es:
                pools = tuple(es.enter_context(p) for p in mk_pools(0))
                phase_b(0, pools)

            # ---- exchange ----
            if world1:
                hsrc = hext_d
            else:
                nc.gpsimd.collective_compute(
                    "AllGather", mybir.AluOpType.bypass,
                    ins=[hloc[:]], outs=[hfull[:]],
                    replica_groups=[list(range(c.n_cores))])
                hsrc = hfull

            # ---- layer 1 ----
            phase_a(1, lambda w0: hsrc[w0 : w0 + CH]
                    .rearrange("w d s p -> d s w p"))
            with contextlib.ExitStack() as es:
                pools = tuple(es.enter_context(p) for p in mk_pools(1))
                phase_b(1, pools)

    nc.compile()
    return nc


# ------------------------------------------------------------ full pipeline

def make_in_maps(cfg, g, cc, xin_cores, meta, convs):
    (c0a, c0p, c1a, c1p) = convs
    maps = []
    for core in range(cfg.n_cores):
        m = dict(
            xin=xin_cores[core],
            iota3=cc["iota3"], ident=cc["ident"],
            meta=meta[core],
        )
        for li, (ca, cp) in ((0, (c0a, c0p)), (1, (c1a, c1p))):
            for s, cv in (("am", ca), ("ph", cp)):
                m[f"Wg{li}_{s}"] = cv["Wg"]
                m[f"Tinv{li}_{s}"] = cv["Tinv"]
                m[f"bcol{li}_{s}"] = cv["bcol"]
                m[f"AR{li}_{s}"] = cv["AR"]
        maps.append(m)
    return maps


def run_pipeline(inputs, cfg, runner):
    g = prep_graph(np.asarray(inputs["src"]), np.asarray(inputs["dst"]), cfg)
    cc = consts_np(cfg, g["TA"] + g["TB"])
    w_am = edge_w_tables(g, inputs["am_exist"], cfg)
    w_ph = edge_w_tables(g, inputs["exist"], cfg)
    dmf = pack_dmf(g, w_am, w_ph, cfg)
    meta = pack_meta(g, dmf)

    conv0a = prep_conv(inputs["W0a"], inputs["al0a"], inputs["ar0a"],
                       inputs["b0a"], False, cfg)
    conv0p = prep_conv(inputs["W0p"], inputs["al0p"], inputs["ar0p"],
                       inputs["b0p"], False, cfg)
    conv1a = prep_conv(inputs["W1a"], inputs["al1a"], inputs["ar1a"],
                       inputs["b1a"], True, cfg)
    conv1p = prep_conv(inputs["W1p"], inputs["al1p"], inputs["ar1p"],
                       inputs["b1p"], True, cfg)

    xT_am = to_xT_tiled(np.asarray(inputs["x_am"]), g, cfg)   # (nw, D, 128)
    xT_ph = to_xT_tiled(np.asarray(inputs["x_ph"]), g, cfg)
    xall = np.stack([xT_am, xT_ph], 2).astype(BF)             # (nw, D, 2, 128)
    xin_cores = [np.ascontiguousarray(xall[c * cfg.wpc:(c + 1) * cfg.wpc])
                 for c in range(cfg.n_cores)]

    nc0 = build_fused(cfg, g["TA"], g["TB"])
    maps = make_in_maps(cfg, g, cc, xin_cores, meta,
                        (conv0a, conv0p, conv1a, conv1p))
    outs = runner(nc0, maps)

    o2 = np.concatenate([np.asarray(o["out2"], np.float32) for o in outs], 0)
    # (nw, 32, 2, 128) -> (n_pad, 32)
    oam = o2[:, :, 0, :].transpose(0, 2, 1).reshape(cfg.n_pad, cfg.fdim)
    oph = o2[:, :, 1, :].transpose(0, 2, 1).reshape(cfg.n_pad, cfg.fdim)
    nid = g["new_id"][: cfg.n_nodes]
    return oam[nid], oph[nid]


# ------------------------------------------------------------ timed runner

def run_layer_timed(nc, in_maps, n_cores, repeats=3):
    import time as _time
    import jax
    from jax.sharding import Mesh, PartitionSpec, NamedSharding
    from concourse import bass2jax
    from jax.experimental.shard_map import shard_map

    bass2jax.install_neuronx_cc_hook()
    part_name = (nc.partition_id_tensor.name
                 if nc.partition_id_tensor is not None else None)
    in_names, out_names, out_avals, zero_outs = [], [], [], []
    for alloc in nc.m.functions[0].allocations:
        if not isinstance(alloc, mybir.MemoryLocationSet):
            continue
        name = alloc.memorylocations[0].name
        if alloc.kind == "ExternalInput":
            if name != part_name:
                in_names.append(name)
        elif alloc.kind == "ExternalOutput":
            out_names.append(name)
            shape = tuple(alloc.tensor_shape)
            dtype = mybir.dt.np(alloc.dtype)
            out_avals.append(jax.core.ShapedArray(shape, dtype))
            zero_outs.append(np.zeros(shape, dtype))
    n_params = len(in_names)
    all_in = list(in_names + out_names)
    if part_name is not None:
        all_in.append(part_name)

    def _body(*args):
        operands = list(args)
        if part_name is not None:
            operands.append(bass2jax.partition_id_tensor())
        outs = bass2jax._bass_exec_p.bind(
            *operands, out_avals=tuple(out_avals), in_names=tuple(all_in),
            out_names=tuple(out_names), lowering_input_output_aliases=(),
            sim_require_finite=True, sim_require_nnan=True, nc=nc)
        return tuple(outs)

    devices = jax.devices()[:n_cores]
    mesh = Mesh(np.asarray(devices), ("core",))
    spec = PartitionSpec("core")
    nin = n_params + len(out_names)
    f = jax.jit(shard_map(_body, mesh=mesh, in_specs=(spec,) * nin,
                          out_specs=(spec,) * len(out_names), check_rep=False))
    concat_in = [np.concatenate([np.asarray(m[nm]) for m in in_maps], 0)
                 for nm in in_names]
    concat_zeros = [np.zeros((n_cores * z.shape[0], *z.shape[1:]), z.dtype)
                    for z in zero_outs]
    sh = NamedSharding(mesh, spec)
    dev_in = [jax.device_put(a, sh) for a in concat_in]
    dev_zero = [jax.device_put(a, sh) for a in concat_zeros]
    outs = f(*dev_in, *dev_zero)
    jax.block_until_ready(outs)
    ts = []
    for _ in range(repeats):
        t0 = _time.perf_counter()
        o2 = f(*dev_in, *dev_zero)
        jax.block_until_ready(o2)
        ts.append(_time.perf_counter() - t0)
    # chained timing: async-dispatch R launches, block once; amortizes RTT
    R = 16
    t0 = _time.perf_counter()
    o2 = None
    for _ in range(R):
        o2 = f(*dev_in, *dev_zero)
    jax.block_until_ready(o2)
    chain = (_time.perf_counter() - t0) / R
    res = []
    for cr in range(n_cores):
        res.append({nm: np.asarray(outs[i]).reshape(n_cores, *out_avals[i].shape)[cr]
                    for i, nm in enumerate(out_names)})
    return res, {"mins": ts, "chain": chain}


def baseline_overhead(n_cores, repeats=5):
    nc = bacc.Bacc("TRN2", target_bir_lowering=False, debug=False)
    x = nc.dram_tensor("x", [P, P], F32, kind="ExternalInput")
    y = nc.dram_tensor("y", [P, P], F32, kind="ExternalOutput")
    with tile.TileContext(nc) as tc:
        with tc.tile_pool(name="p", bufs=1) as p:
            t = p.tile([P, P], F32)
            nc.sync.dma_start(t[:], x[:])
            nc.scalar.mul(t[:], t[:], 2.0)
            nc.sync.dma_start(y[:], t[:])
    nc.compile()
    maps = [{"x": np.zeros((P, P), np.float32)} for _ in range(n_cores)]
    _, tinfo = run_layer_timed(nc, maps, n_cores, repeats=repeats)
    return min(tinfo["mins"]), tinfo["chain"]


# ------------------------------------------------------------ kernel entry

_PERF = {"launch_info": []}


def _hw_runner(cfg, measure):
    from concourse.bass_utils import run_bass_kernel_spmd

    def run(nc, in_maps):
        if measure:
            res, tinfo = run_layer_timed(nc, in_maps, cfg.n_cores, repeats=10)
            _PERF["launch_info"].append(min(tinfo["mins"]))
            _PERF.setdefault("chains", []).append(tinfo["chain"])
            return res
        res = run_bass_kernel_spmd(nc, in_maps,
                                   core_ids=list(range(cfg.n_cores)))
        return res.results
    return run


def kernel(**inputs):
    import os
    cfg = Cfg()
    measure = bool(int(os.environ.get("GAT_MEASURE", "0")))
    res_am, res_ph = run_pipeline(inputs, cfg, _hw_runner(cfg, measure))
    return res_am, res_ph



# revision 3
# speedup vs baseline: 1.1038x; 1.1038x over previous
"""DUPLEX GAT on trn2 — v3.5: replicated-x phase A, fused layer-1
projection, split-table AllGather overlapped with compute, layer-1 er
prebuilt under the collectives.

Design (per core, SPMD over 8 cores; nodes permuted into nw=392 windows of
128, global order g = core*wpc + wl; gather table split per core: table A =
windows wl < wa=25, table B = the rest, so both AllGather outputs are
rank-major contiguous and int16-gather addressable):
  - Phase A0 (replicated): full-node-feature input xT (per-core copy in
    table order, host staged, no x collective on device). Per window:
    g_ps = xw_s.T @ Wg_s (128 ft cols, el-carrying basis so channel f=0 per
    head holds el, f-major) -> interleaved bf16 DRAM row [am|ph] (512B).
    Own-window er via a small pass from xown: er = x @ Wer -> erw0 SBUF.
  - Phase B per dst window (layer li): T = TA+TB tiles of 128 edges
    (A-group tiles then B-group, each sorted by src row for gather
    locality; pad slots gather row 0, masked by S and zero edge weight).
    s0a issues A-table gathers two windows ahead, s0b B-table gathers one
    ahead (head-of-line relief so B1's A-side work runs under the g1B
    collective). S one-hot via iota==dmb on DVE. Layer 0 er in-loop
    (PE transposes -> Act copy -> er matmuls; PE/Act have slack in B0);
    layer 1 er prebuilt into er_sb during the g1 collectives. tb = el
    (Z cols 0:8) + er; lrelu; pz = exp; cb = pz * w_edge; Z *= cb;
    num += S_t.T @ Z_t; den += S_t.T @ pz_t. Epilogue: rec=1/max(den,eps);
    sca = num*rec; PE transpose; h = Tinv.T @ sca; bias + relu/identity.
  - Layer-0 s4 fuses g1 = h @ Wg1ext (132 cols: ft + er): ft -> g1loc DRAM,
    er -> erw1 SBUF. AllGather g1A fires when the first wa windows finish
    (hidden under B0's tail), g1B at the end (hidden under the layer-1 er
    prebuild). Layer 1 gathers from g1A/g1B; out2 written in s4.
"""
import sys
sys.path.insert(0, '/opt/trn_rl_repo')
from dataclasses import dataclass

import numpy as np
import ml_dtypes

import concourse.bass as bass
import concourse.bacc as bacc
import concourse.tile as tile
from concourse import mybir

F32 = mybir.dt.float32
BF16 = mybir.dt.bfloat16
I16 = mybir.dt.int16
P = 128
NEG = 0.2
BF = ml_dtypes.bfloat16


def _patch_drain_split():
    import bass_rust
    from concourse.tile import ScopedClock
    if getattr(tile.TileContext, "_drain_patched_v2", False):
        return

    def patched(self, tick_clock, wait_clock):
        nc = self.nc
        drain_inst = nc.sync.drain()
        wait_clock.add_sem_waits(
            drain_inst.ins, ScopedClock({None: tick_clock.global_clock}))
        si = drain_inst.ins.sync_info
        waits = list(si.on_wait) if si is not None else []
        if len(waits) > 1:
            si.on_wait = waits[:1]
            for i in range(1, len(waits)):
                d2 = nc.sync.drain()
                d2.ins.sync_info = bass_rust.SyncInfo(
                    on_wait=waits[i : i + 1], on_update=[])
        nc.all_engine_barrier()
        popped = nc._tile_sem_poison_stack.pop()
        assert popped is self._sem_poison
        nc.clear_and_free_semaphores(list(self.sems.allocated().values()))
        nc.all_engine_barrier()

    tile.TileContext._drain_and_barrier = patched
    tile.TileContext._drain_patched_v2 = True


_patch_drain_split()


def _patch_loud_ncc():
    import traceback
    from concourse import bass2jax
    if getattr(bass2jax, "_loud_ncc", False):
        return
    bass2jax._loud_ncc = True
    orig = bass2jax.neuronx_cc_hook

    def logged(*a, **k):
        try:
            return orig(*a, **k)
        except BaseException:
            with open("/tmp/ncc_hook_err.log", "a") as f:
                f.write(traceback.format_exc() + "\n")
            raise

    bass2jax.neuronx_cc_hook = logged


_patch_loud_ncc()


@dataclass
class Cfg:
    n_nodes: int = 50000
    n_edges: int = 800000
    n_cores: int = 8
    wpc: int = 49
    wa: int = 25            # local windows in table A (wl < wa)
    heads: int = 4
    fdim: int = 32
    in_dim: int = 128
    gbatch: int = 8         # tiles per dma_gather call (desc-ring limit)
    dma_scratch: int = 16384  # desc ring: 1024 descs

    @property
    def nw(self):
        return self.n_cores * self.wpc

    @property
    def n_pad(self):
        return self.nw * P

    @property
    def wb(self):
        return self.wpc - self.wa

    @property
    def na_rows(self):
        return self.n_cores * self.wa * P      # 25600

    @property
    def nb_rows(self):
        return self.n_cores * self.wb * P      # 24576


# ----------------------------------------------------------------- host prep

def balance_windows(deg, cfg):
    import heapq
    n_pad, nw = cfg.n_pad, cfg.nw
    degp = np.zeros(n_pad, np.int64)
    degp[: len(deg)] = deg
    order = np.argsort(-degp, kind="stable")
    heap = [(0, w, 0) for w in range(nw)]
    heapq.heapify(heap)
    win_of = np.empty(n_pad, np.int32)
    slot_of = np.empty(n_pad, np.int32)
    for node in order:
        load, w, cnt = heapq.heappop(heap)
        win_of[node] = w
        slot_of[node] = cnt
        cnt += 1
        if cnt < P:
            heapq.heappush(heap, (load + degp[node], w, cnt))
    return win_of.astype(np.int64) * P + slot_of


def wrap_idx(idx):
    n = len(idx)
    blk = np.asarray(idx, np.int16).reshape(n // 16, 16).T
    return np.tile(blk, (8, 1))


def prep_graph(src, dst, cfg):
    """Global node order g = core*wpc + wl. Table A = windows with wl < wa
    (row (core*wa + wl)*128 + slot), table B = rest. Per dst window:
    A-group edges first (sorted by src row), then B-group."""
    c = cfg
    deg = np.bincount(dst, minlength=c.n_nodes)
    new_id = balance_windows(deg, c)
    node_at = np.full(c.n_pad, -1, np.int64)
    node_at[new_id] = np.arange(c.n_pad)

    # table-row mapping per global row id
    g_all = np.arange(c.n_pad) // P
    sl_all = np.arange(c.n_pad) % P
    r_all = g_all // c.wpc
    wl_all = g_all % c.wpc
    inA = wl_all < c.wa
    trow = np.where(
        inA,
        (r_all * c.wa + wl_all) * P + sl_all,
        (r_all * c.wb + (wl_all - c.wa)) * P + sl_all).astype(np.int64)

    src_n = new_id[src]
    dst_n = new_id[dst]
    order = np.argsort(dst_n, kind="stable")
    src_s, dst_s = src_n[order], dst_n[order]
    srcA = inA[src_s]
    src_t = trow[src_s]
    win_s = dst_s // P
    bounds = np.searchsorted(win_s, np.arange(c.nw + 1))

    TA = TB = 0
    for g in range(c.nw):
        lo, hi = bounds[g], bounds[g + 1]
        nA = int(srcA[lo:hi].sum())
        nB = int((hi - lo) - nA)
        TA = max(TA, -(-nA // P))
        TB = max(TB, -(-nB // P))
    TA = max(TA, 1)
    TB = max(TB, 1)
    T = TA + TB

    nco, wpc = c.n_cores, c.wpc
    idx16 = np.zeros((nco, wpc, P, T * 8), np.int16)
    dmb = np.full((nco, wpc, P, T), -1000.0, np.float32)
    epos = np.full((nco, wpc, T * P), -1, np.int64)

    for core in range(nco):
        for wl in range(wpc):
            g = core * wpc + wl
            lo, hi = bounds[g], bounds[g + 1]
            rr = src_t[lo:hi]
            dd = dst_s[lo:hi]
            mA = srcA[lo:hi]
            iA = np.where(mA)[0]
            iB = np.where(~mA)[0]
            # sort each group by src table row for gather locality
            iA = iA[np.argsort(rr[iA], kind="stable")]
            iB = iB[np.argsort(rr[iB], kind="stable")]
            la = np.zeros(TA * P, np.int64)      # pad idx 0 (finite row)
            lb = np.zeros(TB * P, np.int64)
            la[: len(iA)] = rr[iA]
            lb[: len(iB)] = rr[iB]
            idx16[core, wl, :, 0 : TA * 8] = wrap_idx(la)
            idx16[core, wl, :, TA * 8 :] = wrap_idx(lb)
            dv = np.full(T * P, -1000.0, np.float32)
            dv[: len(iA)] = (dd[iA] % P).astype(np.float32)
            dv[TA * P : TA * P + len(iB)] = (dd[iB] % P).astype(np.float32)
            dmb[core, wl] = dv.reshape(T, P).T
            ep = np.full(T * P, -1, np.int64)
            ep[: len(iA)] = lo + iA
            ep[TA * P : TA * P + len(iB)] = lo + iB
            epos[core, wl] = ep

    # xT permutation: global window id in table order (A rows then B rows)
    worder = ([r * wpc + wl for r in range(nco) for wl in range(c.wa)]
              + [r * wpc + wl for r in range(nco) for wl in range(c.wa, wpc)])

    return dict(new_id=new_id, node_at=node_at, order=order,
                TA=TA, TB=TB, idx16=idx16, dmb=dmb, epos=epos,
                worder=np.asarray(worder))


def edge_w_tables(g, w_edge, cfg):
    """(nc, wpc, P, T) f32 edge weights in tile layout."""
    ws = np.asarray(w_edge)[g["order"]]
    ep = g["epos"]
    out = np.where(ep >= 0, ws[np.clip(ep, 0, None)], 0.0).astype(np.float32)
    n, w, TP = out.shape
    T = TP // P
    return out.reshape(n, w, T, P).transpose(0, 1, 3, 2).copy()


def pack_dmf(g, w_am, w_ph, cfg):
    """[nc, wpc, 128, 3T] bf16: [0:T) dmb, then per t: (w_am, w_ph)."""
    nco, wpc, _, T = g["dmb"].shape
    dmf = np.zeros((nco, wpc, P, 3 * T), np.float32)
    dmf[..., 0:T] = g["dmb"]
    dmf[..., T::2] = w_am
    dmf[..., T + 1 :: 2] = w_ph
    return dmf.astype(BF)


def prep_conv(W, al, ar, b, mean_heads, cfg):
    H, F = cfg.heads, cfg.fdim
    W = np.asarray(W, np.float64)
    al = np.asarray(al, np.float64)
    ar = np.asarray(ar, np.float64)
    b = np.asarray(b, np.float64)
    T_bd = np.zeros((H * F, H * F))
    for h in range(H):
        a = al[h]
        M = np.concatenate([a[:, None], np.eye(F)[:, : F - 1]], 1)
        Q, _ = np.linalg.qr(M)
        blk = np.concatenate([a[:, None], Q[:, 1:]], 1)
        T_bd[h * F : (h + 1) * F, h * F : (h + 1) * F] = blk
    Tinv = np.linalg.inv(T_bd)
    # f-major column order for the ft block: new col f*H + h <- old h*F + f.
    Wft = (W @ T_bd).reshape(-1, H, F).transpose(0, 2, 1).reshape(-1, H * F)
    # er = x @ Wer directly from original W: Wer[:, h] = W_h @ ar[h]
    Wer = np.zeros((W.shape[0], H))
    for h in range(H):
        Wer[:, h] = W[:, h * F : (h + 1) * F] @ ar[h]
    # Wg ext: [ft (HF) | er (H)] -> 132 cols
    Wg = np.concatenate([Wft, Wer], axis=1)
    out = dict(Wg=np.ascontiguousarray(Wg).astype(BF))
    Tinv_fm = Tinv.reshape(H, F, H * F).transpose(1, 0, 2).reshape(H * F, H * F)
    if mean_heads:
        Mm = np.zeros((H * F, F))
        for h in range(H):
            Mm[h * F : (h + 1) * F] = np.eye(F) / H
        out["Tinv"] = (Tinv_fm @ Mm).astype(BF)
        out["bcol"] = (b.reshape(H, F).mean(0))[:, None].astype(np.float32)
    else:
        out["Tinv"] = Tinv_fm.astype(BF)
        out["bcol"] = b[:, None].astype(np.float32)
    return out


def consts_np(cfg, T):
    # iota3[p, t, n] = n  (t-major S build)
    iota3 = np.tile(np.arange(P, dtype=np.float32)[None, None, :],
                    (P, T, 1)).astype(BF)
    ident = np.eye(P, dtype=np.float32).astype(BF)
    return dict(iota3=iota3, ident=ident)


def pack_meta(g, dmf):
    """[nc, wpc, 128, T*8 + 3T] int16: gather idxs then dmf bits."""
    return np.concatenate([g["idx16"], dmf.view(np.int16)], axis=-1)


def to_xT_tiled(x, g, cfg):
    """x (n_nodes, D) -> permuted transposed tiles (nw, D, 128) f32."""
    n_pad = cfg.n_pad
    D = x.shape[1]
    xp = np.zeros((n_pad, D), np.float32)
    real = g["node_at"] >= 0
    idx = g["node_at"][real]
    keep = idx < cfg.n_nodes
    xp[np.where(real)[0][keep]] = np.asarray(x, np.float32)[idx[keep]]
    return np.ascontiguousarray(xp.reshape(cfg.nw, P, D).transpose(0, 2, 1))


# ------------------------------------------------------------ fused program

def build_fused(cfg, TA, TB):
    """Both GAT layers in one program."""
    c = cfg
    T = TA + TB
    HF = c.heads * c.fdim            # 128
    HE = HF + c.heads                # 132 (ft + er cols)
    NWA = c.n_cores * c.wa           # windows in table A (xT order)
    nc = bacc.Bacc("TRN2", target_bir_lowering=False, debug=False,
                   dynamic_dma_scratch_size=c.dma_scratch)

    def dram_in(name, shape, dt=BF16):
        return nc.dram_tensor(name, list(shape), dt, kind="ExternalInput")

    xT = dram_in("xT", (c.nw, c.in_dim, 2, P))        # full, table order
    xown = dram_in("xown", (c.wpc, c.in_dim, 2, P))   # own windows slice
    Wg = {}
    Tinv = {}
    bcol = {}
    for li in (0, 1):
        for s in ("am", "ph"):
            Wg[(li, s)] = dram_in(f"Wg{li}_{s}", (c.in_dim, HE))
            oc = c.fdim if li else HF
            Tinv[(li, s)] = dram_in(f"Tinv{li}_{s}", (HF, oc))
            bcol[(li, s)] = dram_in(f"bcol{li}_{s}", (oc, 1), F32)
    iota3_d = dram_in("iota3", (P, T, P))
    ident_d = dram_in("ident", (P, P))
    meta_d = dram_in("meta", (c.wpc, P, T * 8 + 3 * T), I16)

    out_d = nc.dram_tensor("out2", [c.wpc, c.fdim, 2, P], F32,
                           kind="ExternalOutput")
    stacks = ("am", "ph")
    CH = 8
    SC = 6
    n_chunk = -(-T // SC)
    assert NWA % CH == 0

    with tile.TileContext(nc) as tc:
        with (
            tc.tile_pool(name="dram", bufs=1, space="DRAM") as dpool,
            tc.tile_pool(name="const", bufs=1) as cpool,
        ):
            g0A = dpool.tile([c.na_rows, 2 * HF], BF16, name="g0A", tag="g0A")
            g0B = dpool.tile([c.nb_rows, 2 * HF], BF16, name="g0B", tag="g0B")
            g1loc = dpool.tile([c.wpc, P, 2 * HF], BF16, name="g1loc",
                               tag="g1loc")
            g1A = dpool.tile([c.na_rows, 2 * HF], BF16, name="g1A",
                             tag="g1A", addr_space="Shared")
            g1B = dpool.tile([c.nb_rows, 2 * HF], BF16, name="g1B",
                             tag="g1B", addr_space="Shared")

            ct = {}
            t = cpool.tile([P, T, P], BF16, name="ct_iota3")
            nc.sync.dma_start(t[:], iota3_d[:])
            ct["iota3"] = t
            t = cpool.tile([P, P], BF16, name="ct_ident")
            nc.sync.dma_start(t[:], ident_d[:])
            ct["ident"] = t
            for li in (0, 1):
                for s in stacks:
                    for nm, hd, dt in [("Wg", Wg[(li, s)], BF16),
                                       ("Tinv", Tinv[(li, s)], BF16),
                                       ("bcol", bcol[(li, s)], F32)]:
                        tt_ = cpool.tile(list(hd.shape), dt,
                                         name=f"ct_{nm}{li}_{s}")
                        nc.sync.dma_start(tt_[:], hd[:])
                        ct[(nm, li, s)] = tt_
            # node-partitioned er, per layer: [128, wpc, 8] bf16
            erw = {li: cpool.tile([P, c.wpc, 2 * c.heads], BF16,
                                  name=f"erw{li}") for li in (0, 1)}
            # per-edge er, prebuilt per layer into idle zones (A0 /
            # collectives); reused buffer across layers
            er_sb = cpool.tile([P, c.wpc, T, 2 * c.heads], BF16,
                               name="er_sb")

            regs = {}

            def _reg(n):
                if n not in regs:
                    regs[n] = nc.gpsimd.to_reg(n)
                return regs[n]

            # ---- own-window er pass (layer 0) ----
            with (
                tc.tile_pool(name="er_x", bufs=3) as perx,
                tc.tile_pool(name="er_ps", bufs=2, space="PSUM") as perps,
            ):
                for wl in range(c.wpc):
                    xo = perx.tile([c.in_dim, 2, P], BF16, name="xo", tag="xo")
                    nc.sync.dma_start(xo[:], xown[wl])
                    e_ps = perps.tile([P, 2 * c.heads], F32, name="e_ps",
                                      tag="eps")
                    for si, s in enumerate(stacks):
                        nc.tensor.matmul(
                            e_ps[:, si * c.heads : (si + 1) * c.heads],
                            xo[:, si, :], ct[("Wg", 0, s)][:, HF:HE],
                            start=True, stop=True)
                    nc.scalar.copy(erw[0][:, wl, :], e_ps[:])

            # ---- phase A0: full table from replicated xT ----
            import contextlib
            with contextlib.ExitStack() as es0:
                pax = es0.enter_context(tc.tile_pool(name="pa_x", bufs=3))
                pag = es0.enter_context(tc.tile_pool(name="pa_g", bufs=3))
                paps = es0.enter_context(
                    tc.tile_pool(name="pa_ps", bufs=4, space="PSUM"))
                for w0 in range(0, c.nw, CH):  # noqa: B007
                    xw = pax.tile([c.in_dim, 2, CH, P], BF16,
                                  name="xw", tag="x")
                    nc.sync.dma_start(
                        xw[:], xT[w0 : w0 + CH].rearrange("w d s p -> d s w p"))
                    gsb = pag.tile([P, CH, 2 * HF], BF16, name="gsb",
                                   tag="gsb")
                    for k in range(CH):
                        g_ps = paps.tile([P, 2, HF], F32, name="g_ps", tag="g")
                        for si, s in enumerate(stacks):
                            nc.tensor.matmul(
                                g_ps[:, si, :], xw[:, si, k, :],
                                ct[("Wg", 0, s)][:, 0:HF],
                                start=True, stop=True)
                        dst = gsb[:, k, :].rearrange(
                            "p (f s h) -> p s f h", s=2, h=c.heads)
                        srcv = g_ps[:].rearrange(
                            "p s (f h) -> p s f h", h=c.heads)
                        if (w0 + k) % 2 == 0:
                            nc.scalar.copy(dst, srcv)
                        else:
                            nc.vector.tensor_copy(dst, srcv)
                    # xT is host-permuted to table order: A rows then B rows
                    if w0 < NWA:
                        dst_ap = g0A[w0 * P : (w0 + CH) * P, :]
                    else:
                        dst_ap = g0B[(w0 - NWA) * P : (w0 - NWA + CH) * P, :]
                    nc.sync.dma_start(
                        dst_ap.rearrange("(w p) d -> p w d", p=P), gsb[:])

            def prebuild_win(li, wl, pm_, ps_, pb_, pt_, pe_):
                """S -> PE transpose -> er matmuls -> er_sb for one window."""
                dmf = pm_.tile([P, 3 * T], I16, name="dmf", tag="dmf")
                nc.sync.dma_start(dmf[:], meta_d[wl][:, T * 8 :])
                dmf_t = dmf[:].bitcast(BF16)
                S = ps_.tile([P, T, P], BF16, name="S", tag="S")
                nc.vector.tensor_tensor(
                    out=S[:], in0=ct["iota3"][:],
                    in1=dmf_t[:, 0:T].unsqueeze(2).broadcast_to([P, T, P]),
                    op=mybir.AluOpType.is_equal)
                STb = pb_.tile([P, T, P], BF16, name="ST", tag="ST")
                for ch in range(n_chunk):
                    t0 = ch * SC
                    t1 = min(T, t0 + SC)
                    st_ps = pt_.tile([P, SC, P], BF16, name="st_ps", tag="st")
                    for t in range(t0, t1):
                        nc.tensor.transpose(st_ps[:, t - t0, :], S[:, t, :],
                                            ct["ident"][:])
                    nc.scalar.copy(STb[:, t0:t1, :],
                                   st_ps[:, 0 : t1 - t0, :])
                er_ps = pe_.tile([P, T, 2 * c.heads], F32,
                                 name="er_ps", tag="erps")
                for t in range(T):
                    nc.tensor.matmul(er_ps[:, t, :], STb[:, t, :],
                                     erw[li][:, wl, :],
                                     start=True, stop=True)
                nc.scalar.copy(er_sb[:, wl, :, :], er_ps[:])

            def mk_pre_pools(li):
                return (
                    tc.tile_pool(name=f"pre{li}_m", bufs=3),
                    tc.tile_pool(name=f"pre{li}_s", bufs=3),
                    tc.tile_pool(name=f"pre{li}_st", bufs=2),
                    tc.tile_pool(name=f"pre{li}_ps", bufs=2, space="PSUM"),
                    tc.tile_pool(name=f"pre{li}_er", bufs=2, space="PSUM"),
                )

            def phase_b(li, pools, chunks=None):
                (pbm, pbz, pbs, pbsm, pbepi,
                 pst, pser, psacc, psden, psmm8) = pools
                last = li == 1
                OC = c.fdim if last else HF
                ODT = F32 if last else BF16
                tabA, tabB = (g0A, g0B) if li == 0 else (g1A, g1B)
                st_meta = {}
                st_z = {}
                st_s = {}
                st_er = {}
                st_pz = {}
                st_acc = {}

                def gath(meta_t, Z, base, ntile, tab):
                    gb = c.gbatch
                    for off in range(0, ntile, gb):
                        nb = min(gb, ntile - off)
                        o = base + off
                        nc.gpsimd.dma_gather(
                            out_ap=Z[:, o : o + nb, :],
                            in_ap=tab[:],
                            idxs_ap=meta_t[:, o * 8 : (o + nb) * 8],
                            num_idxs=nb * P, num_idxs_reg=_reg(nb * P),
                            elem_size=2 * HF)

                def s0a(wl):
                    meta_t = pbm.tile([P, T * 8 + 3 * T], I16,
                                      name="meta", tag="meta")
                    nc.sync.dma_start(meta_t[:], meta_d[wl])
                    Z = pbz.tile([P, T, 2 * HF], BF16, name="Z", tag="Z")
                    gath(meta_t, Z, 0, TA, tabA)
                    st_meta[wl] = meta_t
                    st_z[wl] = Z

                def s0b(wl):
                    gath(st_meta[wl], st_z[wl], TA, TB, tabB)

                def s1(wl):
                    meta_t = st_meta[wl]
                    dmf_t = meta_t[:, T * 8 :].bitcast(BF16)
                    S = pbs.tile([P, T, P], BF16, name="S", tag="S")
                    nc.vector.tensor_tensor(
                        out=S[:],
                        in0=ct["iota3"][:],
                        in1=dmf_t[:, 0:T].unsqueeze(2).broadcast_to([P, T, P]),
                        op=mybir.AluOpType.is_equal)
                    st_s[wl] = S
                    if li == 0:
                        # in-loop er: PE transposes + er matmuls (PE/Act have
                        # slack in B0; prebuild would overload DVE)
                        STb = pbsm.tile([P, T, P], BF16, name="ST", tag="ST")
                        for ch in range(n_chunk):
                            t0 = ch * SC
                            t1 = min(T, t0 + SC)
                            st_ps = pst.tile([P, SC, P], BF16, name="st_ps",
                                             tag="st")
                            for t in range(t0, t1):
                                nc.tensor.transpose(
                                    st_ps[:, t - t0, :], S[:, t, :],
                                    ct["ident"][:])
                            nc.scalar.copy(STb[:, t0:t1, :],
                                           st_ps[:, 0 : t1 - t0, :])
                        er_ps = pser.tile([P, T, 2 * c.heads], F32,
                                          name="er_ps", tag="erps")
                        for t in range(T):
                            nc.tensor.matmul(er_ps[:, t, :], STb[:, t, :],
                                             erw[li][:, wl, :],
                                             start=True, stop=True)
                        st_er[wl] = er_ps

                def s2(wl):
                    Z = st_z[wl]
                    meta_t = st_meta[wl]
                    dmf_t = meta_t[:, T * 8 :].bitcast(BF16)
                    tb = pbsm.tile([P, T, 2 * c.heads], F32, name="tb",
                                   tag="tb")
                    er_src = (st_er.pop(wl)[:] if li == 0
                              else er_sb[:, wl, :, :])
                    nc.vector.tensor_tensor(
                        out=tb[:],
                        in0=Z[:, :, 0 : 2 * c.heads],
                        in1=er_src, op=mybir.AluOpType.add)
                    nc.vector.scalar_tensor_tensor(
                        out=tb[:], in0=tb[:], scalar=NEG, in1=tb[:],
                        op0=mybir.AluOpType.mult, op1=mybir.AluOpType.max)
                    pz = pbsm.tile([P, T, 2 * c.heads], BF16, name="pz",
                                   tag="pz")
                    nc.scalar.activation(pz[:], tb[:],
                                         mybir.ActivationFunctionType.Exp)
                    cb = pbsm.tile([P, T, 2 * c.heads], BF16, name="cb",
                                   tag="cb")
                    nc.vector.tensor_tensor(
                        out=cb[:].rearrange("p t (s h) -> p t s h", s=2),
                        in0=pz[:].rearrange("p t (s h) -> p t s h", s=2),
                        in1=dmf_t[:, T : 3 * T]
                            .rearrange("p (t s) -> p t s", s=2)
                            .unsqueeze(3).broadcast_to([P, T, 2, c.heads]),
                        op=mybir.AluOpType.mult)
                    nc.vector.tensor_tensor(
                        out=Z[:].rearrange("p t (f c) -> p t f c",
                                           c=2 * c.heads),
                        in0=Z[:].rearrange("p t (f c) -> p t f c",
                                           c=2 * c.heads),
                        in1=cb[:].unsqueeze(2)
                            .broadcast_to([P, T, c.fdim, 2 * c.heads]),
                        op=mybir.AluOpType.mult)
                    st_pz[wl] = pz

                def s3(wl):
                    Z = st_z[wl]
                    S = st_s[wl]
                    pz = st_pz[wl]
                    num_t = psacc.tile([P, 2 * HF], F32, name="num", tag="num")
                    den_t = psden.tile([P, 2 * c.heads], F32, name="den",
                                       tag="den")
                    for t in range(T):
                        nc.tensor.matmul(num_t[:], S[:, t, :], Z[:, t, :],
                                         start=(t == 0), stop=(t == T - 1))
                        nc.tensor.matmul(den_t[:], S[:, t, :], pz[:, t, :],
                                         start=(t == 0), stop=(t == T - 1))
                    st_acc[wl] = (num_t, den_t)

                def s4(wl):
                    num_t, den_t = st_acc.pop(wl)
                    num_ps = num_t[:]
                    den_ps = den_t[:]
                    denm = pbepi.tile([P, 2 * c.heads], F32, name="denm",
                                      tag="denm")
                    nc.vector.tensor_scalar(
                        out=denm[:], in0=den_ps, scalar1=1e-9, scalar2=None,
                        op0=mybir.AluOpType.max)
                    rec = pbepi.tile([P, 2 * c.heads], F32, name="rec",
                                     tag="rec")
                    nc.vector.reciprocal(rec[:], denm[:])
                    sca = pbepi.tile([P, 2, HF], BF16, name="sca", tag="sca")
                    numv = num_ps.rearrange("p (f s h) -> p f s h",
                                            s=2, h=c.heads)
                    recv = rec[:].rearrange("p (s h) -> p s h", s=2)
                    h2 = pbepi.tile([OC, 2, P], ODT, name="h2", tag="h2")
                    for si, s in enumerate(stacks):
                        nc.vector.tensor_tensor(
                            out=sca[:, si, :].rearrange("p (f h) -> p f h",
                                                        h=c.heads),
                            in0=numv[:, :, si, :],
                            in1=recv[:, si : si + 1, :]
                                .broadcast_to([P, c.fdim, c.heads]),
                            op=mybir.AluOpType.mult)
                        tr_ps = pst.tile([P, SC, P], BF16, name="st_ps",
                                         tag="st")
                        nc.tensor.transpose(tr_ps[:, 0, :], sca[:, si, :],
                                            ct["ident"][:])
                        scT = pbepi.tile([P, P], BF16, name="scT", tag="scT")
                        nc.scalar.copy(scT[:], tr_ps[:, 0, :])
                        h_ps = psmm8.tile([P, 2, HE], F32, name="mm8",
                                          tag="mm8")
                        nc.tensor.matmul(h_ps[0:OC, 0, 0:P],
                                         ct[("Tinv", li, s)][:], scT[:],
                                         start=True, stop=True)
                        nc.scalar.activation(
                            h2[:, si, :], h_ps[0:OC, 0, 0:P],
                            (mybir.ActivationFunctionType.Identity if last
                             else mybir.ActivationFunctionType.Relu),
                            bias=ct[("bcol", li, s)][:], scale=1.0)
                    if last:
                        nc.sync.dma_start(out_d[wl], h2[:])
                    else:
                        # fused layer-1 projection: g1 = h @ Wg1ext
                        g1_ps = psmm8.tile([P, 2, HE], F32, name="mm8",
                                           tag="mm8")
                        for si, s in enumerate(stacks):
                            nc.tensor.matmul(
                                g1_ps[:, si, :], h2[:, si, :],
                                ct[("Wg", 1, s)][:],
                                start=True, stop=True)
                        nc.scalar.copy(
                            erw[1][:, wl, :].rearrange("p (s h) -> p s h",
                                                       s=2),
                            g1_ps[:, :, HF:HE])
                        g1sb = pbepi.tile([P, 2 * HF], BF16, name="g1sb",
                                          tag="g1sb")
                        nc.scalar.copy(
                            g1sb[:].rearrange("p (f s h) -> p s f h",
                                              s=2, h=c.heads),
                            g1_ps[:, :, 0:HF].rearrange(
                                "p s (f h) -> p s f h", h=c.heads))
                        nc.sync.dma_start(g1loc[wl], g1sb[:])

                def maybe_collective(i):
                    # after s4(i-2): fire AllGather A when own A windows done,
                    # AllGather B at the end
                    if chunks is None:
                        return
                    done = i - 1  # windows [0, done) have s4 issued
                    if done == c.wa and not chunks[0]:
                        chunks[0] = True
                        nc.gpsimd.collective_compute(
                            "AllGather", mybir.AluOpType.bypass,
                            ins=[g1loc[0 : c.wa]], outs=[g1A[:]],
                            replica_groups=[list(range(c.n_cores))])
                    if done == c.wpc and not chunks[1]:
                        chunks[1] = True
                        nc.gpsimd.collective_compute(
                            "AllGather", mybir.AluOpType.bypass,
                            ins=[g1loc[c.wa : c.wpc]], outs=[g1B[:]],
                            replica_groups=[list(range(c.n_cores))])

                s0a(0)
                s0a(1)
                s0b(0)
                for i in range(c.wpc + 2):
                    if i + 2 < c.wpc:
                        s0a(i + 2)
                    if i + 1 < c.wpc:
                        s0b(i + 1)
                    if i < c.wpc:
                        s1(i)
                    if 1 <= i <= c.wpc:
                        s2(i - 1)
                        s3(i - 1)
                    if i >= 2:
                        s4(i - 2)
                        maybe_collective(i)

            def mk_pools(li):
                return (
                    tc.tile_pool(name=f"pb{li}_meta", bufs=7),
                    tc.tile_pool(name=f"pb{li}_z", bufs=6),
                    tc.tile_pool(name=f"pb{li}_s", bufs=5),
                    tc.tile_pool(name=f"pb{li}_small", bufs=4),
                    tc.tile_pool(name=f"pb{li}_epi", bufs=3),
                    tc.tile_pool(name=f"ps{li}_t", bufs=2, space="PSUM"),
                    tc.tile_pool(name=f"ps{li}_er", bufs=2, space="PSUM"),
                    tc.tile_pool(name=f"ps{li}_acc", bufs=2, space="PSUM"),
                    tc.tile_pool(name=f"ps{li}_den", bufs=1, space="PSUM"),
                    tc.tile_pool(name=f"ps{li}_mm8", bufs=1, space="PSUM"),
                )

            with contextlib.ExitStack() as es:
                pools = tuple(es.enter_context(p) for p in mk_pools(0))
                phase_b(0, pools, chunks=[False, False])

            # layer-1 er prebuild drains during the exposed g1 collectives
            with contextlib.ExitStack() as es:
                pre1 = tuple(es.enter_context(p) for p in mk_pre_pools(1))
                for wl in range(c.wpc):
                    prebuild_win(1, wl, *pre1)

            with contextlib.ExitStack() as es:
                pools = tuple(es.enter_context(p) for p in mk_pools(1))
                phase_b(1, pools)

    nc.compile()
    return nc


# ------------------------------------------------------------ full pipeline

def make_in_maps(cfg, g, cc, xall, meta, convs):
    (c0a, c0p, c1a, c1p) = convs
    xperm = np.ascontiguousarray(xall[g["worder"]])
    maps = []
    for core in range(cfg.n_cores):
        m = dict(
            xT=xperm,
            xown=np.ascontiguousarray(
                xall[core * cfg.wpc : (core + 1) * cfg.wpc]),
            iota3=cc["iota3"], ident=cc["ident"],
            meta=meta[core],
        )
        for li, (ca, cp) in ((0, (c0a, c0p)), (1, (c1a, c1p))):
            for s, cv in (("am", ca), ("ph", cp)):
                m[f"Wg{li}_{s}"] = cv["Wg"]
                m[f"Tinv{li}_{s}"] = cv["Tinv"]
                m[f"bcol{li}_{s}"] = cv["bcol"]
        maps.append(m)
    return maps


def run_pipeline(inputs, cfg, runner):
    g = prep_graph(np.asarray(inputs["src"]), np.asarray(inputs["dst"]), cfg)
    cc = consts_np(cfg, g["TA"] + g["TB"])
    w_am = edge_w_tables(g, inputs["am_exist"], cfg)
    w_ph = edge_w_tables(g, inputs["exist"], cfg)
    dmf = pack_dmf(g, w_am, w_ph, cfg)
    meta = pack_meta(g, dmf)

    conv0a = prep_conv(inputs["W0a"], inputs["al0a"], inputs["ar0a"],
                       inputs["b0a"], False, cfg)
    conv0p = prep_conv(inputs["W0p"], inputs["al0p"], inputs["ar0p"],
                       inputs["b0p"], False, cfg)
    conv1a = prep_conv(inputs["W1a"], inputs["al1a"], inputs["ar1a"],
                       inputs["b1a"], True, cfg)
    conv1p = prep_conv(inputs["W1p"], inputs["al1p"], inputs["ar1p"],
                       inputs["b1p"], True, cfg)

    xT_am = to_xT_tiled(np.asarray(inputs["x_am"]), g, cfg)   # (nw, D, 128)
    xT_ph = to_xT_tiled(np.asarray(inputs["x_ph"]), g, cfg)
    xall = np.stack([xT_am, xT_ph], 2).astype(BF)             # (nw, D, 2, 128)

    nc0 = build_fused(cfg, g["TA"], g["TB"])
    maps = make_in_maps(cfg, g, cc, xall, meta,
                        (conv0a, conv0p, conv1a, conv1p))
    outs = runner(nc0, maps)

    o2 = np.concatenate([np.asarray(o["out2"], np.float32) for o in outs], 0)
    # (nw, 32, 2, 128) -> (n_pad, 32)
    oam = o2[:, :, 0, :].transpose(0, 2, 1).reshape(cfg.n_pad, cfg.fdim)
    oph = o2[:, :, 1, :].transpose(0, 2, 1).reshape(cfg.n_pad, cfg.fdim)
    nid = g["new_id"][: cfg.n_nodes]
    return oam[nid], oph[nid]


# ------------------------------------------------------------ timed runner

def run_layer_timed(nc, in_maps, n_cores, repeats=3):
    import time as _time
    import jax
    from jax.sharding import Mesh, PartitionSpec, NamedSharding
    from concourse import bass2jax
    from jax.experimental.shard_map import shard_map

    bass2jax.install_neuronx_cc_hook()
    part_name = (nc.partition_id_tensor.name
                 if nc.partition_id_tensor is not None else None)
    in_names, out_names, out_avals, zero_outs = [], [], [], []
    for alloc in nc.m.functions[0].allocations:
        if not isinstance(alloc, mybir.MemoryLocationSet):
            continue
        name = alloc.memorylocations[0].name
        if alloc.kind == "ExternalInput":
            if name != part_name:
                in_names.append(name)
        elif alloc.kind == "ExternalOutput":
            out_names.append(name)
            shape = tuple(alloc.tensor_shape)
            dtype = mybir.dt.np(alloc.dtype)
            out_avals.append(jax.core.ShapedArray(shape, dtype))
            zero_outs.append(np.zeros(shape, dtype))
    n_params = len(in_names)
    all_in = list(in_names + out_names)
    if part_name is not None:
        all_in.append(part_name)

    def _body(*args):
        operands = list(args)
        if part_name is not None:
            operands.append(bass2jax.partition_id_tensor())
        outs = bass2jax._bass_exec_p.bind(
            *operands, out_avals=tuple(out_avals), in_names=tuple(all_in),
            out_names=tuple(out_names), lowering_input_output_aliases=(),
            sim_require_finite=True, sim_require_nnan=True, nc=nc)
        return tuple(outs)

    devices = jax.devices()[:n_cores]
    mesh = Mesh(np.asarray(devices), ("core",))
    spec = PartitionSpec("core")
    nin = n_params + len(out_names)
    f = jax.jit(shard_map(_body, mesh=mesh, in_specs=(spec,) * nin,
                          out_specs=(spec,) * len(out_names), check_rep=False))
    concat_in = [np.concatenate([np.asarray(m[nm]) for m in in_maps], 0)
                 for nm in in_names]
    concat_zeros = [np.zeros((n_cores * z.shape[0], *z.shape[1:]), z.dtype)
                    for z in zero_outs]
    sh = NamedSharding(mesh, spec)
    dev_in = [jax.device_put(a, sh) for a in concat_in]
    dev_zero = [jax.device_put(a, sh) for a in concat_zeros]
    outs = f(*dev_in, *dev_zero)
    jax.block_until_ready(outs)
    ts = []
    for _ in range(repeats):
        t0 = _time.perf_counter()
        o2 = f(*dev_in, *dev_zero)
        jax.block_until_ready(o2)
        ts.append(_time.perf_counter() - t0)
    # chained timing: async-dispatch R launches, block once; amortizes RTT
    R = 16
    t0 = _time.perf_counter()
    o2 = None
    for _ in range(R):
        o2 = f(*dev_in, *dev_zero)
    jax.block_until_ready(o2)
    chain = (_time.perf_counter() - t0) / R
    res = []
    for cr in range(n_cores):
        res.append({nm: np.asarray(outs[i]).reshape(n_cores, *out_avals[i].shape)[cr]
                    for i, nm in enumerate(out_names)})
    return res, {"mins": ts, "chain": chain}


def baseline_overhead(n_cores, repeats=5):
    nc = bacc.Bacc("TRN2", target_bir_lowering=False, debug=False)
    x = nc.dram_tensor("x", [P, P], F32, kind="ExternalInput")
    y = nc.dram_tensor("y", [P, P], F32, kind="ExternalOutput")
    with tile.TileContext(nc) as tc:
        with tc.tile_pool(name="p", bufs=1) as p:
            t = p.tile([P, P], F32)
            nc.sync.dma_start(t[:], x[:])
            nc.scalar.mul(t[:], t[:], 2.0)
            nc.sync.dma_start(y[:], t[:])
    nc.compile()
    maps = [{"x": np.zeros((P, P), np.float32)} for _ in range(n_cores)]
    _, tinfo = run_layer_timed(nc, maps, n_cores, repeats=repeats)
    return min(tinfo["mins"]), tinfo["chain"]


# ------------------------------------------------------------ kernel entry

_PERF = {"launch_info": []}


def _hw_runner(cfg, measure):
    from concourse.bass_utils import run_bass_kernel_spmd

    def run(nc, in_maps):
        if measure:
            res, tinfo = run_layer_timed(nc, in_maps, cfg.n_cores, repeats=10)
            _PERF["launch_info"].append(min(tinfo["mins"]))
            _PERF.setdefault("chains", []).append(tinfo["chain"])
            return res
        res = run_bass_kernel_spmd(nc, in_maps,
                                   core_ids=list(range(cfg.n_cores)))
        return res.results
    return run


def kernel(**inputs):
    import os
    cfg = Cfg()
    measure = bool(int(os.environ.get("GAT_MEASURE", "0")))
    res_am, res_ph = run_pipeline(inputs, cfg, _hw_runner(cfg, measure))
    return res_am, res_ph
